# revision 2
# baseline (speedup 1.0000x reference)
"""Discrete Hough transform (gather form) on 8 Trainium2 NeuronCores.

Algorithm: for each theta t the reference gathers, for every (rho_r, n),
A_t[n, J] with J = round((rho_r - n*c1)/c2), where A_t is the image (x-sweep)
or its transpose (y-sweep). We decompose J = m_r - s_n - b[r,n] with an
integer shear s_n = floor(n*q), q = c1/c2, and b in {0,1} having the exact
separable threshold form b = [h_n > f_r] (h_n = frac(n*q)).  With the sheared
zero-padded image S[n,j] = Apad[n, j - s_n]:

    params[r, t] = C0[m_r] + C1[m_r - 1]
    C0[j] = sum_n [h_n <= F[j]]   * S[n, j]      F[j] = f_{r: m_r == j}
    C1[j] = sum_n [h_n >  F[j+1]] * S[n, j]

On device, per theta: 4x indirect-DMA builds S (row shifts from a padded
image in DRAM), two fused scalar_tensor_tensor ops build the masked tensors,
a ones-matmul reduces over partitions into PSUM, and a gpsimd ap_gather plus
a strided add assemble the 725 outputs.  Thetas are sorted by shear span and
dealt round-robin to the 8 cores so every core runs an identical program.
"""
import functools
import numpy as np

NUM_R, NUM_T, N = 725, 180, 512
W = H = 512
NCORES = 8
NI = (NUM_T + NCORES - 1) // NCORES  # 23 iterations/core (184 slots, 4 dummy)
ROWBYTES = 1536                       # padded image row length (elements)
APN = 2 * 512 * 1536                  # flat padded img + imgT
CROW_C1 = 1024                        # crow offset of C1 region
CROW_ZERO = 2048                      # crow zero slot
CROW_LEN = 2052
NIDX = 1456                           # 2 * 728 gather indices
NPAD_R = 728


def _exact_indices():
    """J[t,r,n] exactly as the reference computes it (jax on CPU)."""
    import jax
    import jax.numpy as jnp

    cpu = jax.devices("cpu")[0]
    with jax.default_device(cpu):
        dt = jnp.float32
        diag = jnp.sqrt(jnp.asarray(W * W + H * H, dtype=dt))
        thetas = jnp.arange(NUM_T, dtype=dt) * (jnp.pi / NUM_T)
        rhos = jnp.linspace(-diag, diag, NUM_R, dtype=dt)
        cos_t = jnp.cos(thetas)
        sin_t = jnp.sin(thetas)
        nf = jnp.arange(N).astype(dt)
        EPS = 1e-6

        rho_b = rhos[:, None, None]
        cos_b = cos_t[None, :, None]
        sin_b = sin_t[None, :, None]
        n_b = nf[None, None, :]

        sin_safe = jnp.where(jnp.abs(sin_b) < EPS, jnp.asarray(1.0, dt), sin_b)
        y_i = jnp.round((rho_b - n_b * cos_b) / sin_safe).astype(jnp.int32)
        cos_safe = jnp.where(jnp.abs(cos_b) < EPS, jnp.asarray(1.0, dt), cos_b)
        x_i = jnp.round((rho_b - n_b * sin_b) / cos_safe).astype(jnp.int32)
        use_x = jnp.abs(sin_t) >= jnp.abs(cos_t)

        y_i, x_i, use_x = np.asarray(y_i), np.asarray(x_i), np.asarray(use_x)
        cos_np, sin_np, rhos_np = (np.asarray(cos_t), np.asarray(sin_t),
                                   np.asarray(rhos))
    J = np.where(use_x[:, None, None], np.transpose(y_i, (1, 0, 2)),
                 np.transpose(x_i, (1, 0, 2))).astype(np.int64)
    c1 = np.where(use_x, cos_np, sin_np).astype(np.float32)
    c2 = np.where(use_x, sin_np, cos_np).astype(np.float32)
    return J, use_x, c1, c2, rhos_np


def _theta_table(J_t, use_x, c1, c2, rhos):
    """Tables for one theta."""
    n = np.arange(N)
    q = np.float64(c1) / np.float64(c2)
    s = np.floor(n * q).astype(np.int64)
    h = (n * q - s).astype(np.float32)
    valid = (J_t >= 0) & (J_t < H)
    any_valid = valid.any(axis=1)
    p = rhos.astype(np.float64) / np.float64(c2)
    m = np.floor(p + 0.5).astype(np.int64)
    b = m[:, None] - (J_t + s[None, :])
    # threshold fit (optimal cut per r over h-sorted entries, with boundary
    # pseudo-constraints); identical to the validated prototype.
    order = np.argsort(h, kind="stable")
    h_sorted = h[order]
    tap0 = m[:, None] - s[None, :]
    tap1 = tap0 - 1
    in0 = (tap0 >= 0) & (tap0 < H)
    in1 = (tap1 >= 0) & (tap1 < H)
    inv = ~valid & any_valid[:, None]
    req1 = inv & in0 & ~in1
    req0 = inv & in1 & ~in0
    b_req = np.where(req1, 1, np.where(req0, 0, -1))
    tgt = np.where(valid, b, b_req)[:, order]
    care = tgt >= 0
    isone = (tgt == 1) & care
    iszero = (tgt == 0) & care
    ones_pre = np.concatenate([np.zeros((NUM_R, 1), np.int64),
                               np.cumsum(isone, axis=1)], axis=1)
    zeros_pre = np.concatenate([np.zeros((NUM_R, 1), np.int64),
                                np.cumsum(iszero, axis=1)], axis=1)
    cost = ones_pre + (iszero.sum(axis=1, keepdims=True) - zeros_pre)
    kbest = np.argmin(cost, axis=1)
    viol = int(cost[np.arange(NUM_R), kbest][any_valid].sum())
    viol += int((valid & ((b < 0) | (b > 1))).sum())  # outside 2-tap window
    lo = np.where(kbest > 0, h_sorted[np.clip(kbest - 1, 0, N - 1)], -1.0)
    hi = np.where(kbest < N, h_sorted[np.clip(kbest, 0, N - 1)], 1.0)
    f = ((lo + hi) * 0.5).astype(np.float32)
    smin, smax = int(s.min()), int(s.max())
    return dict(q=q, s=s, h=h, m=m, f=f, any_valid=any_valid,
                sweep=0 if use_x else 1, jmin=smin,
                jw=512 + smax - smin, viol=viol)


@functools.lru_cache(maxsize=1)
def _build_all():
    J, use_x, c1, c2, rhos = _exact_indices()
    tabs = [_theta_table(J[t], bool(use_x[t]), c1[t], c2[t], rhos)
            for t in range(NUM_T)]
    jw_all = np.array([tb["jw"] for tb in tabs])
    order = np.argsort(-jw_all, kind="stable")          # descending span
    slots = list(order) + [int(order[-1])] * (NI * NCORES - NUM_T)
    # iteration width: max over the 8 thetas of that iteration, then pad to 4
    jws = []
    for i in range(NI):
        grp = slots[i * NCORES:(i + 1) * NCORES]
        jw = max(tabs[t]["jw"] for t in grp)
        jws.append(min(1023, (jw + 3) // 4 * 4))
    # per-core constant tensors
    soff = np.zeros((NCORES, NI, 128, 4), np.int32)
    hv = np.zeros((NCORES, NI, 128, 4), np.float32)
    frow = np.full((NCORES, NI, 1024), 2.0, np.float32)
    gidx = np.zeros((NCORES, NI, 16, NIDX // 16), np.int16)
    n = np.arange(N)
    for i in range(NI):
        jw = jws[i]
        for c in range(NCORES):
            t = slots[i * NCORES + c]
            tb = tabs[t]
            s, jmin = tb["s"], tb["jmin"]
            off = (tb["sweep"] * (512 * ROWBYTES) + n * ROWBYTES
                   + 512 + jmin - s)
            assert off.min() >= 0 and (512 + jmin - s + jw).max() <= ROWBYTES
            soff[c, i] = off.reshape(4, 128).T.astype(np.int32)
            hv[c, i] = tb["h"].reshape(4, 128).T
            jrel = tb["m"] - jmin
            fr_ok = tb["any_valid"] & (jrel >= 0) & (jrel <= jw)
            frow[c, i, jrel[fr_ok]] = tb["f"][fr_ok]
            idx = np.full(NIDX, CROW_ZERO, np.int64)
            ok0 = tb["any_valid"] & (jrel >= 0) & (jrel < jw)
            ok1 = tb["any_valid"] & (jrel - 1 >= 0) & (jrel - 1 < jw)
            idx[0:2 * NUM_R:2][ok0] = jrel[ok0]
            idx[1:2 * NUM_R:2][ok1] = CROW_C1 + jrel[ok1] - 1
            gidx[c, i] = idx.astype(np.int16).reshape(NIDX // 16, 16).T
    total_viol = sum(tb["viol"] for tb in tabs)
    return dict(jws=tuple(jws), slots=slots, soff=soff, hv=hv, frow=frow,
                gidx=gidx, total_viol=total_viol)


@functools.lru_cache(maxsize=1)
def _compiled_nc(jws):
    import concourse.bass as bass
    import concourse.bacc as bacc
    import concourse.tile as tile
    from concourse import mybir

    f32, i32, i16 = mybir.dt.float32, mybir.dt.int32, mybir.dt.int16
    nc = bacc.Bacc("TRN2", target_bir_lowering=False, debug=False)
    apad = nc.dram_tensor("apad", [APN, 1], f32, kind="ExternalInput")
    soff = nc.dram_tensor("soff", [NI, 128, 4], i32, kind="ExternalInput")
    hv = nc.dram_tensor("hv", [NI, 128, 4], f32, kind="ExternalInput")
    frow = nc.dram_tensor("frow", [NI, 1024], f32, kind="ExternalInput")
    gidx = nc.dram_tensor("gidx", [NI, 16, NIDX // 16], i16,
                          kind="ExternalInput")
    ones = nc.dram_tensor("ones", [128, 16], f32, kind="ExternalInput")
    out = nc.dram_tensor("out", [NI, NPAD_R], f32, kind="ExternalOutput")

    with tile.TileContext(nc) as tc:
        with (
            tc.tile_pool(name="const", bufs=1) as cpool,
            tc.tile_pool(name="small", bufs=2) as sm,
            tc.tile_pool(name="fr", bufs=2) as frp,
            tc.tile_pool(name="s", bufs=2) as sp,
            tc.tile_pool(name="m", bufs=3) as mp,
            tc.tile_pool(name="c", bufs=2) as cp,
            tc.tile_pool(name="ps", bufs=2, space="PSUM") as ps,
        ):
            ones_t = cpool.tile([128, 16], f32)
            nc.sync.dma_start(out=ones_t[:], in_=ones[:])
            for i in range(NI):
                jw = jws[i]
                soff_t = sm.tile([128, 4], i32, tag="soff")
                hv_t = sm.tile([128, 4], f32, tag="hv")
                gi_t = sm.tile([16, NIDX // 16], i16, tag="gi")
                fr_t = frp.tile([128, 1024], f32, tag="fr")
                nc.sync.dma_start(out=soff_t[:], in_=soff[i])
                nc.sync.dma_start(out=hv_t[:], in_=hv[i])
                nc.sync.dma_start(out=gi_t[:], in_=gidx[i])
                nc.sync.dma_start(
                    out=fr_t[:, 0:jw + 1],
                    in_=frow[i:i + 1, 0:jw + 1].to_broadcast((128, jw + 1)))
                psum_t = ps.tile([16, 2048], f32, tag="psum")
                for c in range(4):
                    s_t = sp.tile([128, jw], f32, tag=f"s{c}")
                    nc.gpsimd.indirect_dma_start(
                        out=s_t[:], out_offset=None, in_=apad[:],
                        in_offset=bass.IndirectOffsetOnAxis(
                            ap=soff_t[:, c:c + 1], axis=0))
                    m0 = mp.tile([128, jw], f32, tag="m0")
                    m1 = mp.tile([128, jw], f32, tag="m1")
                    nc.vector.scalar_tensor_tensor(
                        out=m0[:], in0=fr_t[:, 0:jw], scalar=hv_t[:, c:c + 1],
                        in1=s_t[:], op0=mybir.AluOpType.is_ge,
                        op1=mybir.AluOpType.mult)
                    nc.vector.scalar_tensor_tensor(
                        out=m1[:], in0=fr_t[:, 1:jw + 1],
                        scalar=hv_t[:, c:c + 1], in1=s_t[:],
                        op0=mybir.AluOpType.is_lt, op1=mybir.AluOpType.mult)
                    for n0 in range(0, jw, 512):
                        n1 = min(jw, n0 + 512)
                        nc.tensor.matmul(
                            out=psum_t[:, n0:n1], lhsT=ones_t[:],
                            rhs=m0[:, n0:n1], start=(c == 0), stop=(c == 3))
                        nc.tensor.matmul(
                            out=psum_t[:, CROW_C1 + n0:CROW_C1 + n1],
                            lhsT=ones_t[:], rhs=m1[:, n0:n1],
                            start=(c == 0), stop=(c == 3))
                crow = cp.tile([16, CROW_LEN], f32, tag="crow")
                nc.vector.memset(crow[:, CROW_ZERO:CROW_LEN], 0.0)
                nc.scalar.copy(out=crow[:, 0:jw], in_=psum_t[:, 0:jw])
                nc.scalar.copy(out=crow[:, CROW_C1:CROW_C1 + jw],
                               in_=psum_t[:, CROW_C1:CROW_C1 + jw])
                g_t = cp.tile([16, NIDX], f32, tag="g")
                nc.gpsimd.ap_gather(
                    out_ap=g_t[:], in_ap=crow[:], idxs_ap=gi_t[:],
                    channels=16, num_elems=CROW_LEN, d=1, num_idxs=NIDX)
                pr_t = cp.tile([16, NPAD_R], f32, tag="pr")
                nc.vector.tensor_tensor(
                    out=pr_t[:], in0=g_t[:, 0:NIDX:2], in1=g_t[:, 1:NIDX:2],
                    op=mybir.AluOpType.add)
                nc.sync.dma_start(out=out[i:i + 1, :], in_=pr_t[0:1, :])
    nc.compile()
    return nc


def kernel(img: np.ndarray) -> np.ndarray:
    from concourse.bass_utils import run_bass_kernel_spmd

    tabs = _build_all()
    nc = _compiled_nc(tabs["jws"])
    img = np.ascontiguousarray(np.asarray(img, dtype=np.float32))
    apad = np.zeros((2, 512, ROWBYTES), np.float32)
    apad[0, :, 512:1024] = img
    apad[1, :, 512:1024] = img.T
    apad = apad.reshape(APN, 1)
    ones_np = np.ones((128, 16), np.float32)
    in_maps = [dict(apad=apad, soff=tabs["soff"][c], hv=tabs["hv"][c],
                    frow=tabs["frow"][c], gidx=tabs["gidx"][c], ones=ones_np)
               for c in range(NCORES)]
    res = run_bass_kernel_spmd(nc, in_maps, core_ids=list(range(NCORES)))
    params = np.zeros((NUM_R, NUM_T), np.float32)
    slots = tabs["slots"]
    for c in range(NCORES):
        o = res.results[c]["out"]          # [NI, 728]
        for i in range(NI):
            t = slots[i * NCORES + c]
            if i * NCORES + c < NUM_T:
                params[:, t] = o[i, :NUM_R]
    return params


# revision 7
# speedup vs baseline: 1.0066x; 1.0066x over previous
"""Discrete Hough transform (gather form) on 8 Trainium2 NeuronCores.

Algorithm: for each theta t the reference gathers, for every (rho_r, n),
A_t[n, J] with J = round((rho_r - n*c1)/c2), where A_t is the image (x-sweep)
or its transpose (y-sweep). We decompose J = m_r - s_n - b[r,n] with an
integer shear s_n = floor(n*q), q = c1/c2, and b in {0,1} having the exact
separable threshold form b = [h_n > f_r] (h_n = frac(n*q)).  With the sheared
zero-padded image S[n,j] = Apad[n, j - s_n]:

    params[r, t] = C0[m_r] + C1[m_r - 1]
    C0[j] = sum_n [h_n <= F[j]]   * S[n, j]      F[j] = f_{r: m_r == j}
    C1[j] = sum_n [h_n >  F[j+1]] * S[n, j]

On device, per theta: 4x indirect-DMA builds S (row shifts from a padded
image in DRAM), two fused scalar_tensor_tensor ops build the masked tensors,
a ones-matmul reduces over partitions into PSUM, and a gpsimd ap_gather plus
a strided add assemble the 725 outputs.  Thetas are sorted by shear span and
dealt round-robin to the 8 cores so every core runs an identical program.
"""
import functools
import numpy as np

NUM_R, NUM_T, N = 725, 180, 512
W = H = 512
NCORES = 8
NI = (NUM_T + NCORES - 1) // NCORES  # 23 iterations/core (184 slots, 4 dummy)
ROWBYTES = 1536                       # padded image row length (elements)
APN = 2 * 512 * 1536                  # flat padded img + imgT
CROW_C1 = 1024                        # crow offset of C1 region
CROW_ZERO = 2048                      # crow zero slot
CROW_LEN = 2052
NIDX = 1456                           # 2 * 728 gather indices
NPAD_R = 728


def _exact_indices():
    """J[t,r,n] exactly as the reference computes it (jax on CPU)."""
    import jax
    import jax.numpy as jnp

    cpu = jax.devices("cpu")[0]
    with jax.default_device(cpu):
        dt = jnp.float32
        diag = jnp.sqrt(jnp.asarray(W * W + H * H, dtype=dt))
        thetas = jnp.arange(NUM_T, dtype=dt) * (jnp.pi / NUM_T)
        rhos = jnp.linspace(-diag, diag, NUM_R, dtype=dt)
        cos_t = jnp.cos(thetas)
        sin_t = jnp.sin(thetas)
        nf = jnp.arange(N).astype(dt)
        EPS = 1e-6

        rho_b = rhos[:, None, None]
        cos_b = cos_t[None, :, None]
        sin_b = sin_t[None, :, None]
        n_b = nf[None, None, :]

        sin_safe = jnp.where(jnp.abs(sin_b) < EPS, jnp.asarray(1.0, dt), sin_b)
        y_i = jnp.round((rho_b - n_b * cos_b) / sin_safe).astype(jnp.int32)
        cos_safe = jnp.where(jnp.abs(cos_b) < EPS, jnp.asarray(1.0, dt), cos_b)
        x_i = jnp.round((rho_b - n_b * sin_b) / cos_safe).astype(jnp.int32)
        use_x = jnp.abs(sin_t) >= jnp.abs(cos_t)

        y_i, x_i, use_x = np.asarray(y_i), np.asarray(x_i), np.asarray(use_x)
        cos_np, sin_np, rhos_np = (np.asarray(cos_t), np.asarray(sin_t),
                                   np.asarray(rhos))
    J = np.where(use_x[:, None, None], np.transpose(y_i, (1, 0, 2)),
                 np.transpose(x_i, (1, 0, 2))).astype(np.int64)
    c1 = np.where(use_x, cos_np, sin_np).astype(np.float32)
    c2 = np.where(use_x, sin_np, cos_np).astype(np.float32)
    return J, use_x, c1, c2, rhos_np


def _theta_table(J_t, use_x, c1, c2, rhos):
    """Tables for one theta."""
    n = np.arange(N)
    q = np.float64(c1) / np.float64(c2)
    s = np.floor(n * q).astype(np.int64)
    h = (n * q - s).astype(np.float32)
    valid = (J_t >= 0) & (J_t < H)
    any_valid = valid.any(axis=1)
    p = rhos.astype(np.float64) / np.float64(c2)
    m = np.floor(p + 0.5).astype(np.int64)
    b = m[:, None] - (J_t + s[None, :])
    # threshold fit (optimal cut per r over h-sorted entries, with boundary
    # pseudo-constraints); identical to the validated prototype.
    order = np.argsort(h, kind="stable")
    h_sorted = h[order]
    tap0 = m[:, None] - s[None, :]
    tap1 = tap0 - 1
    in0 = (tap0 >= 0) & (tap0 < H)
    in1 = (tap1 >= 0) & (tap1 < H)
    inv = ~valid & any_valid[:, None]
    req1 = inv & in0 & ~in1
    req0 = inv & in1 & ~in0
    b_req = np.where(req1, 1, np.where(req0, 0, -1))
    tgt = np.where(valid, b, b_req)[:, order]
    care = tgt >= 0
    isone = (tgt == 1) & care
    iszero = (tgt == 0) & care
    ones_pre = np.concatenate([np.zeros((NUM_R, 1), np.int64),
                               np.cumsum(isone, axis=1)], axis=1)
    zeros_pre = np.concatenate([np.zeros((NUM_R, 1), np.int64),
                                np.cumsum(iszero, axis=1)], axis=1)
    cost = ones_pre + (iszero.sum(axis=1, keepdims=True) - zeros_pre)
    kbest = np.argmin(cost, axis=1)
    viol = int(cost[np.arange(NUM_R), kbest][any_valid].sum())
    viol += int((valid & ((b < 0) | (b > 1))).sum())  # outside 2-tap window
    lo = np.where(kbest > 0, h_sorted[np.clip(kbest - 1, 0, N - 1)], -1.0)
    hi = np.where(kbest < N, h_sorted[np.clip(kbest, 0, N - 1)], 1.0)
    f = ((lo + hi) * 0.5).astype(np.float32)
    smin, smax = int(s.min()), int(s.max())
    return dict(q=q, s=s, h=h, m=m, f=f, any_valid=any_valid,
                sweep=0 if use_x else 1, jmin=smin,
                jw=512 + smax - smin, viol=viol)


@functools.lru_cache(maxsize=1)
def _build_all():
    J, use_x, c1, c2, rhos = _exact_indices()
    tabs = [_theta_table(J[t], bool(use_x[t]), c1[t], c2[t], rhos)
            for t in range(NUM_T)]
    jw_all = np.array([tb["jw"] for tb in tabs])
    order = np.argsort(-jw_all, kind="stable")          # descending span
    slots = list(order) + [int(order[-1])] * (NI * NCORES - NUM_T)
    # iteration width: max over the 8 thetas of that iteration, then pad to 4
    jws = []
    for i in range(NI):
        grp = slots[i * NCORES:(i + 1) * NCORES]
        jw = max(tabs[t]["jw"] for t in grp)
        jws.append(min(1024, (jw + 3) // 4 * 4))
    # per-core constant tensors
    soff = np.zeros((NCORES, NI, 128, 4), np.int32)
    hv = np.zeros((NCORES, NI, 128, 4), np.float32)
    frow = np.full((NCORES, NI, 1028), 2.0, np.float32)
    gidx = np.zeros((NCORES, NI, 16, NIDX // 16), np.int16)
    n = np.arange(N)
    for i in range(NI):
        jw = jws[i]
        for c in range(NCORES):
            t = slots[i * NCORES + c]
            tb = tabs[t]
            s, jmin = tb["s"], tb["jmin"]
            off = (tb["sweep"] * (512 * ROWBYTES) + n * ROWBYTES
                   + 512 + jmin - s)
            assert off.min() >= 0 and (512 + jmin - s + jw).max() <= ROWBYTES
            soff[c, i] = off.reshape(4, 128).T.astype(np.int32)
            hv[c, i] = tb["h"].reshape(4, 128).T
            jrel = tb["m"] - jmin
            fr_ok = tb["any_valid"] & (jrel >= 0) & (jrel <= jw)
            frow[c, i, jrel[fr_ok]] = tb["f"][fr_ok]
            idx = np.full(NIDX, CROW_ZERO, np.int64)
            ok0 = tb["any_valid"] & (jrel >= 0) & (jrel < jw)
            ok1 = tb["any_valid"] & (jrel - 1 >= 0) & (jrel - 1 < jw)
            idx[0:2 * NUM_R:2][ok0] = jrel[ok0]
            idx[1:2 * NUM_R:2][ok1] = CROW_C1 + jrel[ok1] - 1
            gidx[c, i] = idx.astype(np.int16).reshape(NIDX // 16, 16).T
    total_viol = sum(tb["viol"] for tb in tabs)
    return dict(jws=tuple(jws), slots=slots, soff=soff, hv=hv, frow=frow,
                gidx=gidx, total_viol=total_viol)


@functools.lru_cache(maxsize=1)
def _compiled_nc(jws):
    import concourse.bass as bass
    import concourse.bacc as bacc
    import concourse.tile as tile
    from concourse import mybir

    f32, i32, i16 = mybir.dt.float32, mybir.dt.int32, mybir.dt.int16
    nc = bacc.Bacc("TRN2", target_bir_lowering=False, debug=False)
    apad = nc.dram_tensor("apad", [APN, 1], f32, kind="ExternalInput")
    soff = nc.dram_tensor("soff", [NI, 128, 4], i32, kind="ExternalInput")
    hv = nc.dram_tensor("hv", [NI, 128, 4], f32, kind="ExternalInput")
    frow = nc.dram_tensor("frow", [NI, 1028], f32, kind="ExternalInput")
    gidx = nc.dram_tensor("gidx", [NI, 16, NIDX // 16], i16,
                          kind="ExternalInput")
    ones = nc.dram_tensor("ones", [128, 16], mybir.dt.float32r,
                          kind="ExternalInput")
    out = nc.dram_tensor("out", [NI, NPAD_R], f32, kind="ExternalOutput")

    f32r = mybir.dt.float32r
    with tile.TileContext(nc) as tc:
        with (
            tc.tile_pool(name="const", bufs=1) as cpool,
            tc.tile_pool(name="small", bufs=3) as sm,
            tc.tile_pool(name="fr", bufs=2) as frp,
            tc.tile_pool(name="s", bufs=2) as sp,
            tc.tile_pool(name="m", bufs=4) as mp,
            tc.tile_pool(name="c", bufs=2) as cp,
            tc.tile_pool(name="ps", bufs=2, space="PSUM") as ps,
        ):
            ones_t = cpool.tile([128, 16], f32r)
            nc.sync.dma_start(out=ones_t[:], in_=ones[:])
            for i in range(NI):
                jw = jws[i]
                soff_t = sm.tile([128, 4], i32, tag="soff")
                hv_t = sm.tile([128, 4], f32, tag="hv")
                gi_t = sm.tile([16, NIDX // 16], i16, tag="gi")
                fr_t = frp.tile([128, 1028], f32, tag="fr")
                nc.sync.dma_start(out=soff_t[:], in_=soff[i])
                nc.sync.dma_start(out=hv_t[:], in_=hv[i])
                nc.sync.dma_start(out=gi_t[:], in_=gidx[i])
                nc.sync.dma_start(
                    out=fr_t[:, 0:jw + 1],
                    in_=frow[i:i + 1, 0:jw + 1].to_broadcast((128, jw + 1)))
                psum_t = ps.tile([16, 2048], f32, tag="psum")
                for c in range(4):
                    s_t = sp.tile([128, jw], f32, tag=f"s{c}")
                    nc.gpsimd.indirect_dma_start(
                        out=s_t[:], out_offset=None, in_=apad[:],
                        in_offset=bass.IndirectOffsetOnAxis(
                            ap=soff_t[:, c:c + 1], axis=0))
                    sc = s_t[:]
                    m0 = mp.tile([128, jw], f32r, tag="m0")
                    m1 = mp.tile([128, jw], f32r, tag="m1")
                    nc.vector.scalar_tensor_tensor(
                        out=m0[:], in0=fr_t[:, 0:jw], scalar=hv_t[:, c:c + 1],
                        in1=sc, op0=mybir.AluOpType.is_ge,
                        op1=mybir.AluOpType.mult)
                    nc.vector.scalar_tensor_tensor(
                        out=m1[:], in0=fr_t[:, 1:jw + 1],
                        scalar=hv_t[:, c:c + 1], in1=sc,
                        op0=mybir.AluOpType.is_lt, op1=mybir.AluOpType.mult)
                    for n0 in range(0, jw, 512):
                        n1 = min(jw, n0 + 512)
                        nc.tensor.matmul(
                            out=psum_t[:, n0:n1],
                            lhsT=ones_t[:], rhs=m0[:, n0:n1],
                            start=(c == 0), stop=(c == 3))
                        nc.tensor.matmul(
                            out=psum_t[:, CROW_C1 + n0:CROW_C1 + n1],
                            lhsT=ones_t[:], rhs=m1[:, n0:n1],
                            start=(c == 0), stop=(c == 3))
                crow = cp.tile([16, CROW_LEN], f32, tag="crow")
                nc.vector.memset(crow[:, CROW_ZERO:CROW_LEN], 0.0)
                nc.scalar.copy(out=crow[:, 0:jw], in_=psum_t[:, 0:jw])
                nc.scalar.copy(out=crow[:, CROW_C1:CROW_C1 + jw],
                               in_=psum_t[:, CROW_C1:CROW_C1 + jw])
                g_t = cp.tile([16, NIDX], f32, tag="g")
                nc.gpsimd.ap_gather(
                    out_ap=g_t[:], in_ap=crow[:], idxs_ap=gi_t[:],
                    channels=16, num_elems=CROW_LEN, d=1, num_idxs=NIDX)
                pr_t = cp.tile([16, NPAD_R], f32, tag="pr")
                nc.vector.tensor_tensor(
                    out=pr_t[:], in0=g_t[:, 0:NIDX:2], in1=g_t[:, 1:NIDX:2],
                    op=mybir.AluOpType.add)
                nc.sync.dma_start(out=out[i:i + 1, :], in_=pr_t[0:1, :])
    nc.compile()
    return nc


def kernel(img: np.ndarray) -> np.ndarray:
    from concourse.bass_utils import run_bass_kernel_spmd

    tabs = _build_all()
    nc = _compiled_nc(tabs["jws"])
    img = np.ascontiguousarray(np.asarray(img, dtype=np.float32))
    apad = np.zeros((2, 512, ROWBYTES), np.float32)
    apad[0, :, 512:1024] = img
    apad[1, :, 512:1024] = img.T
    apad = apad.reshape(APN, 1)
    ones_np = np.ones((128, 16), np.float32)
    in_maps = [dict(apad=apad, soff=tabs["soff"][c], hv=tabs["hv"][c],
                    frow=tabs["frow"][c], gidx=tabs["gidx"][c], ones=ones_np)
               for c in range(NCORES)]
    res = run_bass_kernel_spmd(nc, in_maps, core_ids=list(range(NCORES)))
    params = np.zeros((NUM_R, NUM_T), np.float32)
    slots = tabs["slots"]
    for c in range(NCORES):
        o = res.results[c]["out"]          # [NI, 728]
        for i in range(NI):
            t = slots[i * NCORES + c]
            if i * NCORES + c < NUM_T:
                params[:, t] = o[i, :NUM_R]
    return params
